# revision 3
# baseline (speedup 1.0000x reference)
"""Trainium2 Bass kernel: CodebookWrapperLinear (vq-codebook quantized linear).

Computes out[b,s,o] = sum_i x[b,s,i] * w[o,i] where
  w[o, g*GS+j] = (codebook / max|codebook|)[indexes[o,g,j]] * exp(scale[o,g])

Strategy (8-way tensor parallel over out-features):
  - each core owns a 2048-row slice of (indexes, scale) and computes the full
    [8192 tokens x 2048 outs] output block; host concatenates along outs.
  - on-device dequant: with t = idx - 1.5, a 4-entry codebook lookup is an
    exact cubic  c3*t^3 + c2*t^2 + c1*t + c0  (for the antisymmetric codebook
    [-1,-.25,.25,1] this is (t^2 + c1/c3)*t with c3 folded into the
    exp(scale) bias).  3 DVE ops + 1 ACT op per weight element.
  - weights dequantized once into a resident SBUF [K, N] bf16 buffer
    (transposed via the DMA xbar), x is streamed, cast to bf16 by the DMA,
    xbar-transposed per 128-token tile, then a dense bf16 GEMM:
    psum[128 tok, 512 outs] += xT[k,128].T @ wT[k, 512] accumulated over k.
"""

import math

import numpy as np

B, S, IN, OUT, GS = 4, 2048, 4096, 16384, 32
G = IN // GS  # 128
N_CORES = 8
N_SHARD = OUT // N_CORES  # 2048

_BUILD_CACHE = {}


def _fit_cubic(codebook):
    """Exact cubic through (t, cb_norm[i]) for t = i - 1.5, i = 0..3."""
    cb = np.asarray(codebook, np.float64).reshape(-1)
    assert cb.shape == (4,), cb.shape
    cbn = cb / np.clip(np.abs(cb).max(), 1e-8, None)
    t = np.array([-1.5, -0.5, 0.5, 1.5])
    V = np.vander(t, 4, increasing=True)  # [1, t, t^2, t^3]
    c = np.linalg.solve(V, cbn)
    return tuple(float(v) for v in c)


def _build(M, N, K, coefs, n_cores):
    """Emit the Bass program: out[M,N] = x[M,K] @ dequant(idx[N,K], scl[N,G]).T"""
    from concourse import bacc
    import concourse.bass as bass
    import concourse.mybir as mybir
    from concourse.tile import TileContext

    f32 = mybir.dt.float32
    bf16 = mybir.dt.bfloat16
    i32 = mybir.dt.int32
    AOT = mybir.AluOpType
    AFT = mybir.ActivationFunctionType

    c0, c1, c2, c3 = coefs
    antisym = abs(c0) < 1e-9 and abs(c2) < 1e-9 and c3 > 1e-12

    Gn = K // GS  # groups per out row
    OC = N // 128  # out chunks
    MT = M // 128  # token tiles
    KC = K // 128  # k chunks
    NSPLIT = 4 if KC % 4 == 0 else 1
    KCQ = KC // NSPLIT  # k chunks per resident-weight split
    NBW = 512  # n-block width (one PSUM bank)
    NB = N // NBW
    SUB = min(1024, KCQ * 128)  # dequant subtile width (divides KCQ*128)
    SUBC = SUB // 128
    GSUB = SUB // GS

    nc = bacc.Bacc(
        "TRN2", target_bir_lowering=False, debug=False, num_devices=n_cores
    )
    x = nc.dram_tensor("x", [M, K], f32, kind="ExternalInput")
    idx = nc.dram_tensor("idx", [N, K], i32, kind="ExternalInput")
    scl = nc.dram_tensor("scl", [N, Gn], f32, kind="ExternalInput")
    out = nc.dram_tensor("out", [M, N], f32, kind="ExternalOutput")

    with TileContext(nc, num_cores=n_cores) as tc:
        with tc.tile_pool(name="wt", bufs=1) as wt_pool, tc.tile_pool(
            name="es", bufs=1
        ) as es_pool:
            wtq = [
                wt_pool.tile([128, KCQ * N], bf16, name=f"wtq{q}", tag=f"wtq{q}")
                for q in range(NSPLIT)
            ]
            es_all = es_pool.tile([128, OC * Gn], f32, name="es_all")

            # ---------- prep phase: exp(scale), dequant, transpose ----------
            with tc.tile_pool(name="prep", bufs=1) as pp:
                es_bias = math.log(c3) if antisym else 0.0
                ebias_t = es_pool.tile([128, 1], f32, name="ebias")
                nc.vector.memset(ebias_t[:, :], es_bias)
                for oc in range(OC):
                    sclt = pp.tile([128, Gn], f32, tag="sclt", bufs=2, name="sclt")
                    nc.sync.dma_start(sclt[:, :], scl[oc * 128 : (oc + 1) * 128, :])
                    nc.scalar.activation(
                        es_all[:, oc * Gn : (oc + 1) * Gn],
                        sclt[:, :],
                        AFT.Exp,
                        bias=ebias_t[:, :],
                        scale=1.0,
                    )

                for q in range(NSPLIT):
                    for s_i in range(KCQ * 128 // SUB):
                        ks = q * KCQ * 128 + s_i * SUB
                        for oc in range(OC):
                            idxt = pp.tile(
                                [128, SUB], i32, tag="idxt", bufs=3, name="idxt"
                            )
                            nc.sync.dma_start(
                                idxt[:, :],
                                idx[oc * 128 : (oc + 1) * 128, ks : ks + SUB],
                            )
                            tf = pp.tile([128, SUB], f32, tag="dq", bufs=6, name="tf")
                            nc.vector.tensor_scalar(
                                tf[:, :], idxt[:, :], 1.5, None, AOT.subtract
                            )
                            sq = pp.tile([128, SUB], f32, tag="dq", bufs=6, name="sq")
                            nc.scalar.activation(sq[:, :], tf[:, :], AFT.Square)
                            v = pp.tile([128, SUB], f32, tag="dq", bufs=6, name="v")
                            if antisym:
                                # v = (t^2 + c1/c3) * t ; c3 folded into es
                                nc.vector.scalar_tensor_tensor(
                                    v[:, :], sq[:, :], c1 / c3, tf[:, :],
                                    AOT.add, AOT.mult,
                                )
                            elif abs(c3) > 1e-12:
                                p = pp.tile(
                                    [128, SUB], f32, tag="dq", bufs=6, name="p"
                                )
                                nc.vector.scalar_tensor_tensor(
                                    p[:, :], sq[:, :], c1 / c3, tf[:, :],
                                    AOT.add, AOT.mult,
                                )
                                qv = pp.tile(
                                    [128, SUB], f32, tag="dq", bufs=6, name="qv"
                                )
                                nc.vector.tensor_scalar(
                                    qv[:, :], sq[:, :], c2, c0, AOT.mult, AOT.add
                                )
                                nc.vector.scalar_tensor_tensor(
                                    v[:, :], p[:, :], c3, qv[:, :], AOT.mult, AOT.add
                                )
                            else:
                                qv = pp.tile(
                                    [128, SUB], f32, tag="dq", bufs=6, name="qv"
                                )
                                nc.vector.tensor_scalar(
                                    qv[:, :], sq[:, :], c2, c0, AOT.mult, AOT.add
                                )
                                nc.vector.scalar_tensor_tensor(
                                    v[:, :], tf[:, :], c1, qv[:, :], AOT.mult, AOT.add
                                )
                            wb = pp.tile(
                                [128, SUB], bf16, tag="wb", bufs=3, name="wb"
                            )
                            g0 = ks // GS
                            es_sl = es_all[:, oc * Gn + g0 : oc * Gn + g0 + GSUB]
                            v3 = v[:, :].rearrange("p (g s) -> p g s", s=GS)
                            w3 = wb[:, :].rearrange("p (g s) -> p g s", s=GS)
                            es3 = es_sl.rearrange("p (g s) -> p g s", s=1)
                            es3b, _ = bass.broadcast_tensor_aps(es3, v3)
                            nc.vector.tensor_tensor(w3, v3, es3b, AOT.mult)
                            for j in range(SUBC):
                                kc_local = (ks - q * KCQ * 128) // 128 + j
                                nc.sync.dma_start_transpose(
                                    wtq[q][
                                        :,
                                        kc_local * N + oc * 128 :
                                        kc_local * N + oc * 128 + 128,
                                    ],
                                    wb[:, j * 128 : (j + 1) * 128],
                                )

            # ---------- main GEMM ----------
            with tc.tile_pool(name="mm", bufs=1) as mp, tc.tile_pool(
                name="ps", bufs=1, space="PSUM"
            ) as psp:
                for m in range(MT):
                    xb = mp.tile([128, K], bf16, tag="xb", bufs=2, name="xb")
                    # SWDGE casting DMA: f32 DRAM -> bf16 SBUF
                    nc.gpsimd.dma_start(xb[:, :], x[m * 128 : (m + 1) * 128, :])
                    xt = mp.tile([128, K], bf16, tag="xt", bufs=2, name="xt")
                    for j in range(KC):
                        nc.sync.dma_start_transpose(
                            xt[:, j * 128 : (j + 1) * 128],
                            xb[:, j * 128 : (j + 1) * 128],
                        )
                    for n in range(NB):
                        ps = psp.tile([128, NBW], f32, tag="ps", bufs=8, name="ps")
                        for j in range(KC):
                            q, jj = divmod(j, KCQ)
                            nc.tensor.matmul(
                                ps[:, :],
                                xt[:, j * 128 : (j + 1) * 128],
                                wtq[q][:, jj * N + n * NBW : jj * N + n * NBW + NBW],
                                start=(j == 0),
                                stop=(j == KC - 1),
                            )
                        ob = mp.tile([128, NBW], f32, tag="ob", bufs=4, name="ob")
                        nc.any.tensor_copy(ob[:, :], ps[:, :])
                        nc.sync.dma_start(
                            out[m * 128 : (m + 1) * 128, n * NBW : (n + 1) * NBW],
                            ob[:, :],
                        )

    nc.finalize()
    return nc


def get_nc(M, N, K, coefs, n_cores):
    key = (M, N, K, coefs, n_cores)
    if key not in _BUILD_CACHE:
        _BUILD_CACHE[key] = _build(M, N, K, coefs, n_cores)
    return _BUILD_CACHE[key]


def kernel(x, codebook, scale, indexes):
    from concourse import bass_utils

    x = np.asarray(x, dtype=np.float32)
    codebook = np.asarray(codebook, dtype=np.float32)
    scale = np.asarray(scale, dtype=np.float32)
    indexes = np.asarray(indexes, dtype=np.int32)

    Bx, Sx, INx = x.shape
    OUTx = indexes.shape[0]
    M = Bx * Sx
    coefs = _fit_cubic(codebook)

    xm = np.ascontiguousarray(x.reshape(M, INx))
    idx2 = np.ascontiguousarray(indexes.reshape(OUTx, INx))
    scl2 = np.ascontiguousarray(scale.reshape(OUTx, INx // GS))

    n_shard = OUTx // N_CORES
    nc = get_nc(M, n_shard, INx, coefs, N_CORES)

    in_maps = []
    for c in range(N_CORES):
        in_maps.append(
            {
                "x": xm,
                "idx": idx2[c * n_shard : (c + 1) * n_shard],
                "scl": scl2[c * n_shard : (c + 1) * n_shard],
            }
        )
    res = bass_utils.run_bass_kernel_spmd(
        nc, in_maps, core_ids=list(range(N_CORES))
    )
    out = np.concatenate(
        [res.results[c]["out"] for c in range(N_CORES)], axis=1
    )
    return out.reshape(Bx, Sx, OUTx)


# revision 5
# speedup vs baseline: 1.5025x; 1.5025x over previous
"""Trainium2 Bass kernel: CodebookWrapperLinear (vq-codebook quantized linear).

Computes out[b,s,o] = sum_i x[b,s,i] * w[o,i] where
  w[o, g*GS+j] = (codebook / max|codebook|)[indexes[o,g,j]] * exp(scale[o,g])

Strategy (8-way tensor parallel over out-features):
  - each core owns a 2048-row slice of (indexes, scale) and computes the full
    [8192 tokens x 2048 outs] output block; host concatenates along outs.
  - on-device dequant: with t = idx - 1.5, a 4-entry codebook lookup is an
    exact cubic  c3*t^3 + c2*t^2 + c1*t + c0  (for the antisymmetric codebook
    [-1,-.25,.25,1] this is (t^2 + c1/c3)*t with c3 folded into the
    exp(scale) bias).  3 DVE ops + 1 ACT op per weight element.
  - weights dequantized once into a resident SBUF [K, N] bf16 buffer
    (transposed via the DMA xbar), x is streamed, cast to bf16 by the DMA,
    xbar-transposed per 128-token tile, then a dense bf16 GEMM:
    psum[128 tok, 512 outs] += xT[k,128].T @ wT[k, 512] accumulated over k.
"""

import math

import numpy as np

B, S, IN, OUT, GS = 4, 2048, 4096, 16384, 32
G = IN // GS  # 128
N_CORES = 8
N_SHARD = OUT // N_CORES  # 2048

_BUILD_CACHE = {}


def _fit_cubic(codebook):
    """Exact cubic through (t, cb_norm[i]) for t = i - 1.5, i = 0..3."""
    cb = np.asarray(codebook, np.float64).reshape(-1)
    assert cb.shape == (4,), cb.shape
    cbn = cb / np.clip(np.abs(cb).max(), 1e-8, None)
    t = np.array([-1.5, -0.5, 0.5, 1.5])
    V = np.vander(t, 4, increasing=True)  # [1, t, t^2, t^3]
    c = np.linalg.solve(V, cbn)
    return tuple(float(v) for v in c)


def _build(M, N, K, coefs, n_cores):
    """Emit the Bass program: out[M,N] = x[M,K] @ dequant(idx[N,K], scl[N,G]).T"""
    from concourse import bacc
    import concourse.bass as bass
    import concourse.mybir as mybir
    from concourse.tile import TileContext

    f32 = mybir.dt.float32
    bf16 = mybir.dt.bfloat16
    i32 = mybir.dt.int32
    AOT = mybir.AluOpType
    AFT = mybir.ActivationFunctionType

    c0, c1, c2, c3 = coefs
    antisym = abs(c0) < 1e-9 and abs(c2) < 1e-9 and c3 > 1e-12

    Gn = K // GS  # groups per out row
    OC = N // 128  # out chunks
    MT = M // 128  # token tiles
    KC = K // 128  # k chunks
    NSPLIT = 4 if KC % 4 == 0 else 1
    KCQ = KC // NSPLIT  # k chunks per resident-weight split
    NBW = 512  # n-block width (one PSUM bank)
    NB = N // NBW
    SUB = min(1024, KCQ * 128)  # dequant subtile width (divides KCQ*128)
    SUBC = SUB // 128
    GSUB = SUB // GS

    nc = bacc.Bacc(
        "TRN2", target_bir_lowering=False, debug=False, num_devices=n_cores
    )
    x = nc.dram_tensor("x", [M, K], f32, kind="ExternalInput")
    idx = nc.dram_tensor("idx", [N, K], i32, kind="ExternalInput")
    scl = nc.dram_tensor("scl", [N, Gn], f32, kind="ExternalInput")
    out = nc.dram_tensor("out", [M, N], f32, kind="ExternalOutput")

    with TileContext(nc, num_cores=n_cores) as tc:
        with tc.tile_pool(name="wt", bufs=1) as wt_pool, tc.tile_pool(
            name="es", bufs=1
        ) as es_pool:
            wtq = [
                wt_pool.tile([128, KCQ * N], bf16, name=f"wtq{q}", tag=f"wtq{q}")
                for q in range(NSPLIT)
            ]
            es_all = es_pool.tile([128, OC * Gn], f32, name="es_all")

            # ---------- prep phase: exp(scale), dequant, transpose ----------
            with tc.tile_pool(name="prep", bufs=1) as pp:
                es_bias = math.log(c3) if antisym else 0.0
                ebias_t = es_pool.tile([128, 1], f32, name="ebias")
                nc.vector.memset(ebias_t[:, :], es_bias)
                for oc in range(OC):
                    sclt = pp.tile([128, Gn], f32, tag="sclt", bufs=2, name="sclt")
                    nc.sync.dma_start(sclt[:, :], scl[oc * 128 : (oc + 1) * 128, :])
                    nc.scalar.activation(
                        es_all[:, oc * Gn : (oc + 1) * Gn],
                        sclt[:, :],
                        AFT.Exp,
                        bias=ebias_t[:, :],
                        scale=1.0,
                    )

                for q in range(NSPLIT):
                    for s_i in range(KCQ * 128 // SUB):
                        ks = q * KCQ * 128 + s_i * SUB
                        for oc in range(OC):
                            idxt = pp.tile(
                                [128, SUB], i32, tag="idxt", bufs=3, name="idxt"
                            )
                            nc.sync.dma_start(
                                idxt[:, :],
                                idx[oc * 128 : (oc + 1) * 128, ks : ks + SUB],
                            )
                            tf = pp.tile([128, SUB], f32, tag="dq", bufs=6, name="tf")
                            nc.vector.tensor_scalar(
                                tf[:, :], idxt[:, :], 1.5, None, AOT.subtract
                            )
                            sq = pp.tile([128, SUB], f32, tag="dq", bufs=6, name="sq")
                            nc.scalar.activation(sq[:, :], tf[:, :], AFT.Square)
                            v = pp.tile([128, SUB], f32, tag="dq", bufs=6, name="v")
                            if antisym:
                                # v = (t^2 + c1/c3) * t ; c3 folded into es
                                nc.vector.scalar_tensor_tensor(
                                    v[:, :], sq[:, :], c1 / c3, tf[:, :],
                                    AOT.add, AOT.mult,
                                )
                            elif abs(c3) > 1e-12:
                                p = pp.tile(
                                    [128, SUB], f32, tag="dq", bufs=6, name="p"
                                )
                                nc.vector.scalar_tensor_tensor(
                                    p[:, :], sq[:, :], c1 / c3, tf[:, :],
                                    AOT.add, AOT.mult,
                                )
                                qv = pp.tile(
                                    [128, SUB], f32, tag="dq", bufs=6, name="qv"
                                )
                                nc.vector.tensor_scalar(
                                    qv[:, :], sq[:, :], c2, c0, AOT.mult, AOT.add
                                )
                                nc.vector.scalar_tensor_tensor(
                                    v[:, :], p[:, :], c3, qv[:, :], AOT.mult, AOT.add
                                )
                            else:
                                qv = pp.tile(
                                    [128, SUB], f32, tag="dq", bufs=6, name="qv"
                                )
                                nc.vector.tensor_scalar(
                                    qv[:, :], sq[:, :], c2, c0, AOT.mult, AOT.add
                                )
                                nc.vector.scalar_tensor_tensor(
                                    v[:, :], tf[:, :], c1, qv[:, :], AOT.mult, AOT.add
                                )
                            wb = pp.tile(
                                [128, SUB], bf16, tag="wb", bufs=3, name="wb"
                            )
                            g0 = ks // GS
                            es_sl = es_all[:, oc * Gn + g0 : oc * Gn + g0 + GSUB]
                            v3 = v[:, :].rearrange("p (g s) -> p g s", s=GS)
                            w3 = wb[:, :].rearrange("p (g s) -> p g s", s=GS)
                            es3 = es_sl.rearrange("p (g s) -> p g s", s=1)
                            es3b, _ = bass.broadcast_tensor_aps(es3, v3)
                            nc.vector.tensor_tensor(w3, v3, es3b, AOT.mult)
                            # one 3D-output xbar transpose for all SUBC chunks:
                            # wtq3[p, j, t] = wb[t, j*128+p]
                            kc0 = (ks - q * KCQ * 128) // 128
                            wtq3 = wtq[q][:, :].rearrange("p (j n) -> p j n", n=N)
                            nc.sync.dma_start_transpose(
                                wtq3[:, kc0 : kc0 + SUBC, oc * 128 : oc * 128 + 128],
                                wb[:, :],
                            )

            # ---------- main GEMM ----------
            with tc.tile_pool(name="mm", bufs=1) as mp, tc.tile_pool(
                name="ps", bufs=1, space="PSUM"
            ) as psp:
                for m in range(MT):
                    xb = mp.tile([128, K], bf16, tag="xb", bufs=2, name="xb")
                    # SWDGE casting DMA: f32 DRAM -> bf16 SBUF
                    nc.gpsimd.dma_start(xb[:, :], x[m * 128 : (m + 1) * 128, :])
                    xt = mp.tile([128, K], bf16, tag="xt", bufs=3, name="xt")
                    nc.sync.dma_start_transpose(
                        xt[:, :].rearrange("p (j t) -> p j t", t=128), xb[:, :]
                    )
                    for n in range(NB):
                        ps = psp.tile([128, NBW], f32, tag="ps", bufs=8, name="ps")
                        for j in range(KC):
                            q, jj = divmod(j, KCQ)
                            nc.tensor.matmul(
                                ps[:, :],
                                xt[:, j * 128 : (j + 1) * 128],
                                wtq[q][:, jj * N + n * NBW : jj * N + n * NBW + NBW],
                                start=(j == 0),
                                stop=(j == KC - 1),
                            )
                        ob = mp.tile([128, NBW], f32, tag="ob", bufs=4, name="ob")
                        nc.any.tensor_copy(ob[:, :], ps[:, :])
                        nc.sync.dma_start(
                            out[m * 128 : (m + 1) * 128, n * NBW : (n + 1) * NBW],
                            ob[:, :],
                        )

    nc.finalize()
    return nc


def get_nc(M, N, K, coefs, n_cores):
    key = (M, N, K, coefs, n_cores)
    if key not in _BUILD_CACHE:
        _BUILD_CACHE[key] = _build(M, N, K, coefs, n_cores)
    return _BUILD_CACHE[key]


def kernel(x, codebook, scale, indexes):
    from concourse import bass_utils

    x = np.asarray(x, dtype=np.float32)
    codebook = np.asarray(codebook, dtype=np.float32)
    scale = np.asarray(scale, dtype=np.float32)
    indexes = np.asarray(indexes, dtype=np.int32)

    Bx, Sx, INx = x.shape
    OUTx = indexes.shape[0]
    M = Bx * Sx
    coefs = _fit_cubic(codebook)

    xm = np.ascontiguousarray(x.reshape(M, INx))
    idx2 = np.ascontiguousarray(indexes.reshape(OUTx, INx))
    scl2 = np.ascontiguousarray(scale.reshape(OUTx, INx // GS))

    n_shard = OUTx // N_CORES
    nc = get_nc(M, n_shard, INx, coefs, N_CORES)

    in_maps = []
    for c in range(N_CORES):
        in_maps.append(
            {
                "x": xm,
                "idx": idx2[c * n_shard : (c + 1) * n_shard],
                "scl": scl2[c * n_shard : (c + 1) * n_shard],
            }
        )
    res = bass_utils.run_bass_kernel_spmd(
        nc, in_maps, core_ids=list(range(N_CORES))
    )
    out = np.concatenate(
        [res.results[c]["out"] for c in range(N_CORES)], axis=1
    )
    return out.reshape(Bx, Sx, OUTx)
